# revision 36
# baseline (speedup 1.0000x reference)
"""Trainium2 Bass kernel for nn_DotAttentionUnit.

Reference computation (per batch b):
    h_mul[p,q,h] = hq[q,h] * hp[p,h]
    s_w = tanh(h_mul @ W.T)            # [p,q,v]
    s[p,q] = s_w . v_w                 # reduce over v
    a = softmax(s, axis=q)
    out[p,h] = sum_q a[p,q] * hq[q,h]

Shapes: B=4, LQ=256, LP=256, H=512, V=512.

Sharding: pure data parallel over (b, p-block): 8 cores = 4 batches x 2
p-blocks of 128. Each core computes out[b, pblk:pblk+128, :]. No
collectives.

Per-core device algorithm (PE-bound, fp16 matmul operands with fp32 PSUM
accumulation; fp16 mantissa ~ TF32, keeps rel err ~1e-4):
  for p in 0..127:
    scaled[k]  = hqT[k] * hpT[k][:, p]     (2 on Pool, 1 on ACT, 1 on Pool)
    psum[m]    = sum_k scaled[k][:,m*128:].T @ WT[k]  (PE, 8 matmuls N=512)
    tw         = tanh(psum)                (ACT, one [128,1024] op)
    sc         = tw * vw                   (DVE, one wide fp16 mul)
    scores[:, :, p] = reduce(sc)           (DVE, one fused wide reduce)
  epilogue (x2 chunks of 64 p-rows, first chunk overlapped mid-loop):
  PE-transpose scores chunk -> exp+sum (ACT, no max shift needed: |s| is
  small) -> transpose exp -> exp^T @ hq -> scale rows by 1/sum -> DMA out.
"""

import numpy as np

B, LQ, LP, H, V = 4, 256, 256, 512, 512
NCORES = 8
PB = 128  # p rows per core
KH = H // 128  # 4 contraction tiles
MQ = LQ // 128  # 2 q tiles
EPI_CHUNK = 64

_CACHED_NC = None


def _build_nc(repeat=1):
    from contextlib import ExitStack

    import concourse.bass as bass
    import concourse.mybir as mybir
    import concourse.tile as tile
    from concourse import bacc
    from concourse.masks import make_identity

    f32 = mybir.dt.float32
    f16 = mybir.dt.float16
    AF = mybir.ActivationFunctionType

    nc = bacc.Bacc("TRN2", target_bir_lowering=False, debug=False)

    # host pre-arranges all inputs into the exact SBUF layouts so every
    # DMA is one contiguous >=1KB run per partition (fewest descriptors)
    hqT_d = nc.dram_tensor("hqT", [128, KH * LQ], f16, kind="ExternalInput")
    hpT_d = nc.dram_tensor("hpT", [128, KH * PB], f32, kind="ExternalInput")
    WT_d = nc.dram_tensor("WT", [128, KH * V], f16, kind="ExternalInput")
    vwb_d = nc.dram_tensor("vwb", [128, MQ * V], f16, kind="ExternalInput")
    hq_d = nc.dram_tensor("hq", [128, MQ * H], f16, kind="ExternalInput")
    out_d = nc.dram_tensor("out", [PB, H], f32, kind="ExternalOutput")

    with tile.TileContext(nc) as tc, ExitStack() as ctx:
        consts = ctx.enter_context(tc.tile_pool(name="consts", bufs=1))
        scaled_pool = ctx.enter_context(tc.tile_pool(name="scaled", bufs=4))
        tanh_pool = ctx.enter_context(tc.tile_pool(name="tanh", bufs=4))
        scratch_pool = ctx.enter_context(tc.tile_pool(name="scratch", bufs=3))
        epi = ctx.enter_context(tc.tile_pool(name="epi", bufs=2))
        psum_main = ctx.enter_context(
            tc.tile_pool(name="psmain", bufs=2, space="PSUM")
        )
        psum_tp = ctx.enter_context(tc.tile_pool(name="pstp", bufs=2, space="PSUM"))
        psum_out = ctx.enter_context(tc.tile_pool(name="psout", bufs=2, space="PSUM"))

        # Startup: both the HWDGE issue path and the DMA transfer path are
        # single serialized devices, so use one combined DMA per tensor
        # ordered by first-use: hqT (gates preps), hpT, WT k0/k1, WT k2/k3.
        wz = consts.tile([128, 128], f16, name="wz")
        nc.vector.memset(wz[:], 0.0)
        hqT_s3 = consts.tile([128, KH, LQ], f16, name="hqT")
        hpT_s3 = consts.tile([128, KH, PB], f32, name="hpT")
        WT_s3 = consts.tile([128, KH, V], f16, name="WT")
        nc.sync.dma_start(
            hqT_s3[:], hqT_d.ap().rearrange("p (k q) -> p k q", k=KH)
        )
        nc.scalar.dma_start(
            hpT_s3[:], hpT_d.ap().rearrange("p (k q) -> p k q", k=KH)
        )
        WT_r3 = WT_d.ap().rearrange("p (k v) -> p k v", k=KH)
        nc.sync.dma_start(WT_s3[:, 0:2, :], WT_r3[:, 0:2, :])
        nc.sync.dma_start(WT_s3[:, 2:4, :], WT_r3[:, 2:4, :])
        vw_s = consts.tile([128, MQ * V], f16)
        nc.gpsimd.dma_start(vw_s[:], vwb_d.ap())
        hq_s = consts.tile([128, MQ, H], f16)
        nc.sync.dma_start(
            hq_s[:], hq_d.ap().rearrange("p (m h) -> p m h", m=MQ)
        )
        hqT_s = [hqT_s3[:, k, :] for k in range(KH)]
        hpT_s = [hpT_s3[:, k, :] for k in range(KH)]
        WT_s = [WT_s3[:, k, :] for k in range(KH)]
        ident = consts.tile([128, 128], f32)
        make_identity(nc, ident[:])
        # scores[q, m, p]: column p filled per main-loop iteration
        scores = consts.tile([128, MQ, PB], f32)

        # PE warmup: dummy matmuls on a zeroed tile fill the otherwise-idle
        # input-DMA window (small-N so overshoot past data-ready is small)
        wps = psum_tp.tile([128, V], f32, tag="tp")
        N_WARM = 47
        for i in range(N_WARM):
            nc.tensor.matmul(
                wps[:, :128], wz[:], wz[:], start=(i == 0), stop=(i == N_WARM - 1)
            )
        wtr = consts.tile([128, 128], f32, name="wtr")

        def epilogue_chunk(c0, csz):
            """softmax over q + attention output for p-rows [c0, c0+csz)."""
            # no max-subtraction: |s| is bounded well inside fp32 exp range
            # for this problem; softmax is shift-invariant so this matches
            # the stable-softmax reference up to rounding. exp reads the
            # transposed scores straight from PSUM (ScalarE sits next to
            # PSUM), skipping an SBUF bounce; the m=0 e-transpose overlaps
            # the m=1 exp
            e_t = epi.tile([csz, LQ], f32, name=f"e_t{c0}", tag="e_t")
            ssum = epi.tile([csz, MQ], f32, name=f"ssum{c0}", tag="ssum")
            eT = epi.tile([128, MQ, csz], f16, name=f"eT{c0}", tag="eT")
            for m in range(MQ):
                pst = psum_tp.tile([csz, 128], f32, tag="tp")
                nc.tensor.transpose(
                    pst[:], scores[:, m, c0 : c0 + csz], ident[:]
                )
                nc.scalar.activation(
                    e_t[:, bass.ts(m, 128)], pst[:],
                    AF.Exp, accum_out=ssum[:, m : m + 1],
                )
                pet = psum_tp.tile([128, csz], f32, tag="tp")
                nc.tensor.transpose(
                    pet[:], e_t[:, bass.ts(m, 128)], ident[:csz, :csz]
                )
                nc.vector.tensor_copy(eT[:, m, :], pet[:])
            ssum_t = epi.tile([csz, 1], f32, name=f"ssumt{c0}", tag="ssumt")
            nc.vector.reduce_sum(
                ssum_t[:], ssum[:], axis=mybir.AxisListType.X
            )
            rcp = epi.tile([csz, 1], f32, name=f"rcp{c0}", tag="rcp")
            nc.vector.reciprocal(rcp[:], ssum_t[:])
            out_ps = psum_out.tile([csz, H], f32, tag="outps")
            for m in range(MQ):
                nc.tensor.matmul(
                    out_ps[:],
                    eT[:, m, :],
                    hq_s[:, m, :],
                    start=(m == 0),
                    stop=(m == MQ - 1),
                )
            out_s = epi.tile([csz, H], f32, name=f"out_s{c0}", tag="out_s")
            nc.scalar.activation(out_s[:], out_ps[:], AF.Copy, scale=rcp[:])
            nc.sync.dma_start(out_d.ap()[c0 : c0 + csz, :], out_s[:])

        for p in range(PB * repeat):
            p = p % PB
            if p == 2:
                nc.vector.tensor_copy(wtr[:], wps[:, :128])
            scaled = [
                scaled_pool.tile([128, LQ], f16, name=f"sc{k}_{p}", tag=f"scl{k}")
                for k in range(KH)
            ]
            for k in range(KH):
                # steady state: k=2 on ACT, rest on Pool. For the first few
                # p, ACT is still issuing input DMAs and Pool's serial preps
                # would starve the PE — run those preps on the idle DVE
                # (fp16 single-src tensor_scalar is 4x-mode there, ~127ns)
                if p < 6 or k < 2:
                    nc.vector.tensor_scalar_mul(
                        scaled[k][:], hqT_s[k][:], hpT_s[k][:, p : p + 1]
                    )
                else:
                    nc.gpsimd.tensor_scalar_mul(
                        scaled[k][:], hqT_s[k][:], hpT_s[k][:, p : p + 1]
                    )
            ps = psum_main.tile([128, MQ * V], f32, tag="ps")
            for m in range(MQ):
                for k in range(KH):
                    nc.tensor.matmul(
                        ps[:, m * V : (m + 1) * V],
                        scaled[k][:, bass.ts(m, 128)],
                        WT_s[k][:],
                        start=(k == 0),
                        stop=(k == KH - 1),
                    )
            tw = tanh_pool.tile([128, MQ * V], f16, tag="tw")
            sc = scratch_pool.tile([128, MQ, V], f16, tag="sc")
            if p < PB - 2:
                nc.scalar.activation(tw[:], ps[:], AF.Tanh)
                nc.vector.tensor_mul(
                    sc[:].rearrange("p m v -> p (m v)"), tw[:], vw_s[:]
                )
                for m in range(MQ):
                    trash = scratch_pool.tile([128, V], f16, tag=f"tr{m}")
                    nc.vector.tensor_scalar(
                        trash[:], sc[:, m, :], 0.0, 0.0,
                        op0=mybir.AluOpType.add,
                        op1=mybir.AluOpType.add,
                        accum_out=scores[:, m, p : p + 1],
                    )
            else:
                # tail latency: split by m so DVE starts on m=0 while ACT
                # still computes m=1's tanh; m=1's reduce rides ACT so the
                # two half-chains finish in parallel
                for m in range(MQ):
                    nc.scalar.activation(
                        tw[:, m * V : (m + 1) * V],
                        ps[:, m * V : (m + 1) * V],
                        AF.Tanh,
                    )
                    nc.vector.tensor_mul(
                        sc[:, m, :], tw[:, m * V : (m + 1) * V],
                        vw_s[:, m * V : (m + 1) * V],
                    )
                    trash = scratch_pool.tile([128, V], f16, tag=f"tr{m}")
                    nc.vector.tensor_scalar(
                        trash[:], sc[:, m, :], 0.0, 0.0,
                        op0=mybir.AluOpType.add,
                        op1=mybir.AluOpType.add,
                        accum_out=scores[:, m, p : p + 1],
                    )
            if (p + 1) % EPI_CHUNK == 0:
                epilogue_chunk(p + 1 - EPI_CHUNK, EPI_CHUNK)

    nc.compile()
    return nc


def get_nc():
    global _CACHED_NC
    if _CACHED_NC is None:
        _CACHED_NC = _build_nc()
    return _CACHED_NC


def make_in_maps(hq, hp, W, v_w):
    hq = np.asarray(hq, dtype=np.float32)
    hp = np.asarray(hp, dtype=np.float32)
    W = np.asarray(W, dtype=np.float32)
    v_w = np.asarray(v_w, dtype=np.float32)
    def to_sbuf_layout(arr_kpx, inner):
        """[K*128, inner] -> [128, K*inner] matching sbuf [part, k, inner]."""
        k = arr_kpx.shape[0] // 128
        return np.ascontiguousarray(
            arr_kpx.reshape(k, 128, inner).transpose(1, 0, 2).reshape(128, -1)
        )

    WT = to_sbuf_layout(np.ascontiguousarray(W.T), V).astype(np.float16)
    vw1 = v_w.reshape(1, V).astype(np.float16)
    vwb = np.ascontiguousarray(
        np.broadcast_to(np.tile(vw1, (1, MQ)), (128, MQ * V))
    )
    in_maps = []
    for c in range(NCORES):
        b = c // 2
        pb = (c % 2) * PB
        in_maps.append(
            {
                "hqT": to_sbuf_layout(
                    np.ascontiguousarray(hq[b].T), LQ
                ).astype(np.float16),
                "hpT": to_sbuf_layout(
                    np.ascontiguousarray(hp[b, pb : pb + PB].T), PB
                ),
                "WT": WT,
                "vwb": vwb,
                "hq": to_sbuf_layout(
                    np.ascontiguousarray(hq[b]), H
                ).astype(np.float16),
            }
        )
    return in_maps


def gather_out(results):
    out = np.empty((B, LP, H), np.float32)
    for c in range(NCORES):
        b = c // 2
        pb = (c % 2) * PB
        out[b, pb : pb + PB] = results[c]["out"]
    return out


def kernel(hq, hp, W, v_w):
    from concourse.bass_utils import run_bass_kernel_spmd

    nc = get_nc()
    in_maps = make_in_maps(hq, hp, W, v_w)
    res = run_bass_kernel_spmd(nc, in_maps, core_ids=list(range(NCORES)))
    return gather_out(res.results)



# revision 37
# speedup vs baseline: 1.0016x; 1.0016x over previous
"""Trainium2 Bass kernel for nn_DotAttentionUnit.

Reference computation (per batch b):
    h_mul[p,q,h] = hq[q,h] * hp[p,h]
    s_w = tanh(h_mul @ W.T)            # [p,q,v]
    s[p,q] = s_w . v_w                 # reduce over v
    a = softmax(s, axis=q)
    out[p,h] = sum_q a[p,q] * hq[q,h]

Shapes: B=4, LQ=256, LP=256, H=512, V=512.

Sharding: pure data parallel over (b, p-block): 8 cores = 4 batches x 2
p-blocks of 128. Each core computes out[b, pblk:pblk+128, :]. No
collectives.

Per-core device algorithm (PE-bound, fp16 matmul operands with fp32 PSUM
accumulation; fp16 mantissa ~ TF32, keeps rel err ~1e-4):
  for p in 0..127:
    scaled[k]  = hqT[k] * hpT[k][:, p]     (2 on Pool, 1 on ACT, 1 on Pool)
    psum[m]    = sum_k scaled[k][:,m*128:].T @ WT[k]  (PE, 8 matmuls N=512)
    tw         = tanh(psum)                (ACT, one [128,1024] op)
    sc         = tw * vw                   (DVE, one wide fp16 mul)
    scores[:, :, p] = reduce(sc)           (DVE, one fused wide reduce)
  epilogue (x2 chunks of 64 p-rows, first chunk overlapped mid-loop):
  PE-transpose scores chunk -> exp+sum (ACT, no max shift needed: |s| is
  small) -> transpose exp -> exp^T @ hq -> scale rows by 1/sum -> DMA out.
"""

import numpy as np

B, LQ, LP, H, V = 4, 256, 256, 512, 512
NCORES = 8
PB = 128  # p rows per core
KH = H // 128  # 4 contraction tiles
MQ = LQ // 128  # 2 q tiles
EPI_CHUNK = 64

_CACHED_NC = None


def _build_nc(repeat=1):
    from contextlib import ExitStack

    import concourse.bass as bass
    import concourse.mybir as mybir
    import concourse.tile as tile
    from concourse import bacc
    from concourse.masks import make_identity

    f32 = mybir.dt.float32
    f16 = mybir.dt.float16
    AF = mybir.ActivationFunctionType

    nc = bacc.Bacc("TRN2", target_bir_lowering=False, debug=False)

    # host pre-arranges all inputs into the exact SBUF layouts so every
    # DMA is one contiguous >=1KB run per partition (fewest descriptors)
    hqT_d = nc.dram_tensor("hqT", [128, KH * LQ], f16, kind="ExternalInput")
    hpT_d = nc.dram_tensor("hpT", [128, KH * PB], f16, kind="ExternalInput")
    WT_d = nc.dram_tensor("WT", [128, KH * V], f16, kind="ExternalInput")
    vwb_d = nc.dram_tensor("vwb", [128, MQ * V], f16, kind="ExternalInput")
    hq_d = nc.dram_tensor("hq", [128, MQ * H], f16, kind="ExternalInput")
    out_d = nc.dram_tensor("out", [PB, H], f32, kind="ExternalOutput")

    with tile.TileContext(nc) as tc, ExitStack() as ctx:
        consts = ctx.enter_context(tc.tile_pool(name="consts", bufs=1))
        scaled_pool = ctx.enter_context(tc.tile_pool(name="scaled", bufs=4))
        tanh_pool = ctx.enter_context(tc.tile_pool(name="tanh", bufs=4))
        scratch_pool = ctx.enter_context(tc.tile_pool(name="scratch", bufs=3))
        epi = ctx.enter_context(tc.tile_pool(name="epi", bufs=2))
        psum_main = ctx.enter_context(
            tc.tile_pool(name="psmain", bufs=2, space="PSUM")
        )
        psum_tp = ctx.enter_context(tc.tile_pool(name="pstp", bufs=2, space="PSUM"))
        psum_out = ctx.enter_context(tc.tile_pool(name="psout", bufs=2, space="PSUM"))

        # Startup: both the HWDGE issue path and the DMA transfer path are
        # single serialized devices, so use one combined DMA per tensor
        # ordered by first-use: hqT (gates preps), hpT, WT k0/k1, WT k2/k3.
        wz = consts.tile([128, 128], f16, name="wz")
        nc.vector.memset(wz[:], 0.0)
        hqT_s3 = consts.tile([128, KH, LQ], f16, name="hqT")
        hpT_s3 = consts.tile([128, KH, PB], f16, name="hpT")
        WT_s3 = consts.tile([128, KH, V], f16, name="WT")
        nc.sync.dma_start(
            hqT_s3[:], hqT_d.ap().rearrange("p (k q) -> p k q", k=KH)
        )
        nc.scalar.dma_start(
            hpT_s3[:], hpT_d.ap().rearrange("p (k q) -> p k q", k=KH)
        )
        WT_r3 = WT_d.ap().rearrange("p (k v) -> p k v", k=KH)
        nc.sync.dma_start(WT_s3[:, 0:2, :], WT_r3[:, 0:2, :])
        nc.sync.dma_start(WT_s3[:, 2:4, :], WT_r3[:, 2:4, :])
        vw_s = consts.tile([128, MQ * V], f16)
        nc.gpsimd.dma_start(vw_s[:], vwb_d.ap())
        hq_s = consts.tile([128, MQ, H], f16)
        nc.sync.dma_start(
            hq_s[:], hq_d.ap().rearrange("p (m h) -> p m h", m=MQ)
        )
        # tensor_scalar needs an f32 scalar operand; hpT ships as f16 to
        # halve its transfer on the serialized DMA device, upconvert once
        hpT_f3 = consts.tile([128, KH, PB], f32, name="hpTf")
        nc.vector.tensor_copy(hpT_f3[:], hpT_s3[:])
        hqT_s = [hqT_s3[:, k, :] for k in range(KH)]
        hpT_s = [hpT_f3[:, k, :] for k in range(KH)]
        WT_s = [WT_s3[:, k, :] for k in range(KH)]
        ident = consts.tile([128, 128], f32)
        make_identity(nc, ident[:])
        # scores[q, m, p]: column p filled per main-loop iteration
        scores = consts.tile([128, MQ, PB], f32)

        # PE warmup: dummy matmuls on a zeroed tile fill the otherwise-idle
        # input-DMA window (small-N so overshoot past data-ready is small)
        wps = psum_tp.tile([128, V], f32, tag="tp")
        N_WARM = 47
        for i in range(N_WARM):
            nc.tensor.matmul(
                wps[:, :128], wz[:], wz[:], start=(i == 0), stop=(i == N_WARM - 1)
            )
        wtr = consts.tile([128, 128], f32, name="wtr")

        def epilogue_chunk(c0, csz):
            """softmax over q + attention output for p-rows [c0, c0+csz)."""
            # no max-subtraction: |s| is bounded well inside fp32 exp range
            # for this problem; softmax is shift-invariant so this matches
            # the stable-softmax reference up to rounding. exp reads the
            # transposed scores straight from PSUM (ScalarE sits next to
            # PSUM), skipping an SBUF bounce; the m=0 e-transpose overlaps
            # the m=1 exp
            e_t = epi.tile([csz, LQ], f32, name=f"e_t{c0}", tag="e_t")
            ssum = epi.tile([csz, MQ], f32, name=f"ssum{c0}", tag="ssum")
            eT = epi.tile([128, MQ, csz], f16, name=f"eT{c0}", tag="eT")
            for m in range(MQ):
                pst = psum_tp.tile([csz, 128], f32, tag="tp")
                nc.tensor.transpose(
                    pst[:], scores[:, m, c0 : c0 + csz], ident[:]
                )
                nc.scalar.activation(
                    e_t[:, bass.ts(m, 128)], pst[:],
                    AF.Exp, accum_out=ssum[:, m : m + 1],
                )
                pet = psum_tp.tile([128, csz], f32, tag="tp")
                nc.tensor.transpose(
                    pet[:], e_t[:, bass.ts(m, 128)], ident[:csz, :csz]
                )
                nc.vector.tensor_copy(eT[:, m, :], pet[:])
            ssum_t = epi.tile([csz, 1], f32, name=f"ssumt{c0}", tag="ssumt")
            nc.vector.reduce_sum(
                ssum_t[:], ssum[:], axis=mybir.AxisListType.X
            )
            rcp = epi.tile([csz, 1], f32, name=f"rcp{c0}", tag="rcp")
            nc.vector.reciprocal(rcp[:], ssum_t[:])
            out_ps = psum_out.tile([csz, H], f32, tag="outps")
            for m in range(MQ):
                nc.tensor.matmul(
                    out_ps[:],
                    eT[:, m, :],
                    hq_s[:, m, :],
                    start=(m == 0),
                    stop=(m == MQ - 1),
                )
            out_s = epi.tile([csz, H], f32, name=f"out_s{c0}", tag="out_s")
            nc.scalar.activation(out_s[:], out_ps[:], AF.Copy, scale=rcp[:])
            nc.sync.dma_start(out_d.ap()[c0 : c0 + csz, :], out_s[:])

        for p in range(PB * repeat):
            p = p % PB
            if p == 2:
                nc.vector.tensor_copy(wtr[:], wps[:, :128])
            scaled = [
                scaled_pool.tile([128, LQ], f16, name=f"sc{k}_{p}", tag=f"scl{k}")
                for k in range(KH)
            ]
            for k in range(KH):
                # steady state: k=2 on ACT, rest on Pool. For the first few
                # p, ACT is still issuing input DMAs and Pool's serial preps
                # would starve the PE — run those preps on the idle DVE
                # (fp16 single-src tensor_scalar is 4x-mode there, ~127ns)
                if p < 6 or k < 2:
                    nc.vector.tensor_scalar_mul(
                        scaled[k][:], hqT_s[k][:], hpT_s[k][:, p : p + 1]
                    )
                else:
                    nc.gpsimd.tensor_scalar_mul(
                        scaled[k][:], hqT_s[k][:], hpT_s[k][:, p : p + 1]
                    )
            ps = psum_main.tile([128, MQ * V], f32, tag="ps")
            for m in range(MQ):
                for k in range(KH):
                    nc.tensor.matmul(
                        ps[:, m * V : (m + 1) * V],
                        scaled[k][:, bass.ts(m, 128)],
                        WT_s[k][:],
                        start=(k == 0),
                        stop=(k == KH - 1),
                    )
            tw = tanh_pool.tile([128, MQ * V], f16, tag="tw")
            sc = scratch_pool.tile([128, MQ, V], f16, tag="sc")
            if p < PB - 2:
                nc.scalar.activation(tw[:], ps[:], AF.Tanh)
                nc.vector.tensor_mul(
                    sc[:].rearrange("p m v -> p (m v)"), tw[:], vw_s[:]
                )
                for m in range(MQ):
                    trash = scratch_pool.tile([128, V], f16, tag=f"tr{m}")
                    nc.vector.tensor_scalar(
                        trash[:], sc[:, m, :], 0.0, 0.0,
                        op0=mybir.AluOpType.add,
                        op1=mybir.AluOpType.add,
                        accum_out=scores[:, m, p : p + 1],
                    )
            else:
                # tail latency: split by m so DVE starts on m=0 while ACT
                # still computes m=1's tanh; m=1's reduce rides ACT so the
                # two half-chains finish in parallel
                for m in range(MQ):
                    nc.scalar.activation(
                        tw[:, m * V : (m + 1) * V],
                        ps[:, m * V : (m + 1) * V],
                        AF.Tanh,
                    )
                    nc.vector.tensor_mul(
                        sc[:, m, :], tw[:, m * V : (m + 1) * V],
                        vw_s[:, m * V : (m + 1) * V],
                    )
                    trash = scratch_pool.tile([128, V], f16, tag=f"tr{m}")
                    nc.vector.tensor_scalar(
                        trash[:], sc[:, m, :], 0.0, 0.0,
                        op0=mybir.AluOpType.add,
                        op1=mybir.AluOpType.add,
                        accum_out=scores[:, m, p : p + 1],
                    )
            if (p + 1) % EPI_CHUNK == 0:
                epilogue_chunk(p + 1 - EPI_CHUNK, EPI_CHUNK)

    nc.compile()
    return nc


def get_nc():
    global _CACHED_NC
    if _CACHED_NC is None:
        _CACHED_NC = _build_nc()
    return _CACHED_NC


def make_in_maps(hq, hp, W, v_w):
    hq = np.asarray(hq, dtype=np.float32)
    hp = np.asarray(hp, dtype=np.float32)
    W = np.asarray(W, dtype=np.float32)
    v_w = np.asarray(v_w, dtype=np.float32)
    def to_sbuf_layout(arr_kpx, inner):
        """[K*128, inner] -> [128, K*inner] matching sbuf [part, k, inner]."""
        k = arr_kpx.shape[0] // 128
        return np.ascontiguousarray(
            arr_kpx.reshape(k, 128, inner).transpose(1, 0, 2).reshape(128, -1)
        )

    WT = to_sbuf_layout(np.ascontiguousarray(W.T), V).astype(np.float16)
    vw1 = v_w.reshape(1, V).astype(np.float16)
    vwb = np.ascontiguousarray(
        np.broadcast_to(np.tile(vw1, (1, MQ)), (128, MQ * V))
    )
    in_maps = []
    for c in range(NCORES):
        b = c // 2
        pb = (c % 2) * PB
        in_maps.append(
            {
                "hqT": to_sbuf_layout(
                    np.ascontiguousarray(hq[b].T), LQ
                ).astype(np.float16),
                "hpT": to_sbuf_layout(
                    np.ascontiguousarray(hp[b, pb : pb + PB].T), PB
                ).astype(np.float16),
                "WT": WT,
                "vwb": vwb,
                "hq": to_sbuf_layout(
                    np.ascontiguousarray(hq[b]), H
                ).astype(np.float16),
            }
        )
    return in_maps


def gather_out(results):
    out = np.empty((B, LP, H), np.float32)
    for c in range(NCORES):
        b = c // 2
        pb = (c % 2) * PB
        out[b, pb : pb + PB] = results[c]["out"]
    return out


def kernel(hq, hp, W, v_w):
    from concourse.bass_utils import run_bass_kernel_spmd

    nc = get_nc()
    in_maps = make_in_maps(hq, hp, W, v_w)
    res = run_bass_kernel_spmd(nc, in_maps, core_ids=list(range(NCORES)))
    return gather_out(res.results)

